# revision 12
# baseline (speedup 1.0000x reference)
"""Trainium2 Bass kernel for ContinuousAttention (self-keyed RoPE attention,
strictly-causal masked scores, no softmax).

Reference computation (B=2, NH=16, T=2048, N=256, fp32):
    QR = rope(Q)                      # interleaved-pair RoPE, freqs quantized in pairs
    S  = QR @ QR^T                    # per (b, h); K input is unused by the module
    O  = (S * strict_causal_mask) @ V

Sharding: 32 (b*nh) heads over 8 NeuronCores, 4 heads per core; no
communication.  Each core runs an identical program on its head slice.

v4 design (fp16 matmul operands, fp32 PSUM accumulation):
  - Microbench facts: fp16 matmul streams at ~0.42ns/moving-col regardless of
    stationary reuse (LDWEIGHTS fully pipelined, no per-instruction cost), so
    minimize moving columns and dependency stalls.
  - RoPE is computed on the HOST (f32, then cast fp16): removes the rope
    tables (2MB DMA), the pair-swapped Q copy (halves input DMA) and the
    vector-engine RoPE chain that delayed dependent matmuls.
  - mm1 row-major descending: strip j's moving range is exactly [128j, T) in
    <=512 chunks; with back-to-front segment DMAs, row 15 starts after one
    64KB transfer, killing the cold-start bubble.
  - mm2 computes O^T (n-major) with V stationary and 512-wide strip moving
    operands; one PSUM bank per pass, drains + output DMA spread throughout.
    Host transposes O^T back.
"""

import math
import sys

import numpy as np

if "/opt/trn_rl_repo" not in sys.path:
    sys.path.insert(0, "/opt/trn_rl_repo")

import concourse.bass as bass
import concourse.mybir as mybir
import concourse.tile as tile
from concourse.bass_utils import run_bass_kernel_spmd

B, NH, T, N = 2, 16, 2048, 256
THETA = 2 ** 16
N_CORES = 8
H_PER_CORE = (B * NH) // N_CORES

F32 = mybir.dt.float32
FP16 = mybir.dt.float16
MULT = mybir.AluOpType.mult
HF = np.float16


def _split_overloaded_waits(nc, max_waits=1):
    """walrus in this container rejects >1 sync-wait per instruction; move
    extra waits onto preceding same-engine NoOps (semantically identical)."""
    n_split = 0
    for f in nc.m.functions:
        for bb in f.blocks:
            new_list = []
            changed = False
            for ins in bb.instructions:
                si = getattr(ins, "sync_info", None)
                if si is not None and len(si.on_wait) > max_waits:
                    waits = list(si.on_wait)
                    extra, keep = waits[:-max_waits], waits[-max_waits:]
                    k = 0
                    while extra:
                        chunk, extra = extra[:max_waits], extra[max_waits:]
                        nop = mybir.InstNoOp(
                            name=f"{ins.name}_wsplit{k}", ins=[], outs=[]
                        )
                        nop.engine = ins.engine
                        nop.sync_info = mybir.SyncInfo(on_wait=chunk, on_update=[])
                        new_list.append(nop)
                        k += 1
                    ins.sync_info = mybir.SyncInfo(
                        on_wait=keep, on_update=list(si.on_update)
                    )
                    changed = True
                    n_split += 1
                new_list.append(ins)
            if changed:
                bb.instructions = new_list
    return n_split


def rope_tables(t=T, n=N, dtype=np.float32):
    """cos table and sign-folded sin table, natural (t, n) layout."""
    idx = np.floor(np.arange(n, dtype=dtype) / dtype(2.0)) * dtype(2.0)
    freqs = (
        dtype(1.0) / (dtype(THETA) ** (idx / dtype(n))) / dtype(2.0 * math.pi)
    ).astype(dtype)
    phases = np.arange(t, dtype=dtype)[:, None] * freqs[None, :]
    ph = (phases % dtype(1.0)) * dtype(2.0 * math.pi)
    cos = np.cos(ph).astype(dtype)
    sin = np.sin(ph).astype(dtype)
    sin_a = sin.copy()
    sin_a[:, 0::2] *= dtype(-1.0)  # fold the rotate-pair sign into sin
    return cos, sin_a


def _desc_segs(t):
    """Descending DMA/compute segments: two 256-wide at the top (fast first
    matmul), then 512-wide."""
    segs = []
    hi = t
    for w in (256, 256):
        segs.append((hi - w, hi))
        hi -= w
    while hi > 0:
        segs.append((hi - 512, hi))
        hi -= 512
    return segs


def build_nc(h_per_core=H_PER_CORE, t=T, n=N, waitsplit=True):
    assert n == 256 and t % 512 == 0
    nt = t // 128  # 128-row strips
    nbk = t // 512  # 512-col output banks
    nc = bass.Bass("TRN2", target_bir_lowering=False, debug=False)

    # q holds host-side-ROPED Q, transposed to (n, t)
    q = nc.dram_tensor("q", [h_per_core, n, t], FP16, kind="ExternalInput").ap()
    # v pre-swizzled on host to [128, t//128, n] (partition = t mod 128)
    v = nc.dram_tensor(
        "v", [h_per_core, 128, nt, n], FP16, kind="ExternalInput"
    ).ap()
    # O^T layout: o[h, nb, p, t] = O[t, nb*128 + p]; host transposes back
    o = nc.dram_tensor("o", [h_per_core, 2, 128, t], F32, kind="ExternalOutput").ap()

    with tile.TileContext(nc) as tc:
        with (
            tc.tile_pool(name="const", bufs=1) as cpool,
            tc.tile_pool(name="qrt", bufs=2) as qrtpool,
            tc.tile_pool(name="strips", bufs=2) as strippool,
            tc.tile_pool(name="vh", bufs=2) as vpool,
            tc.tile_pool(name="oht", bufs=2) as ohtpool,
            tc.tile_pool(name="sps", bufs=5, space="PSUM") as spool,
            tc.tile_pool(name="ops", bufs=3, space="PSUM") as opool,
        ):
            # strict mask in (s, t) orientation: keep iff free > partition
            mask = cpool.tile([128, 128], F32)
            nc.gpsimd.memset(mask, 1.0)
            nc.gpsimd.affine_select(
                out=mask,
                in_=mask,
                compare_op=mybir.AluOpType.is_ge,
                fill=0.0,
                base=-1,
                pattern=[[1, 128]],
                channel_multiplier=-1,
            )

            cp = 0  # writeback engine round-robin

            def wb_copy(dst, src):
                nonlocal cp
                if cp % 2 == 0:
                    nc.vector.tensor_copy(out=dst, in_=src)
                else:
                    nc.scalar.copy(out=dst, in_=src)
                cp += 1

            for h in range(h_per_core):
                # ---- segmented input DMA, back-to-front, parallel queues ----
                qrt = [
                    qrtpool.tile([128, t], FP16, tag=f"qrt{c}", name=f"qrt{c}")
                    for c in range(2)
                ]
                segs = _desc_segs(t)
                if h == 0:
                    # split the first segment 4 ways across idle dispatch
                    # queues so row 15 can start ~1us sooner
                    lo, hi = segs[0]
                    mid = (lo + hi) // 2
                    nc.sync.dma_start(
                        out=qrt[0][:, lo:mid], in_=q[h][0:128, lo:mid]
                    )
                    nc.scalar.dma_start(
                        out=qrt[0][:, mid:hi], in_=q[h][0:128, mid:hi]
                    )
                    nc.gpsimd.dma_start(
                        out=qrt[1][:, lo:mid], in_=q[h][128:256, lo:mid]
                    )
                    nc.gpsimd.dma_start(
                        out=qrt[1][:, mid:hi], in_=q[h][128:256, mid:hi]
                    )
                    segs = segs[1:]
                for lo, hi in segs:
                    tsl = slice(lo, hi)
                    nc.sync.dma_start(
                        out=qrt[0][:, tsl], in_=q[h][0:128, tsl]
                    )
                    nc.gpsimd.dma_start(
                        out=qrt[1][:, tsl], in_=q[h][128:256, tsl]
                    )

                vh = vpool.tile([128, nt * n], FP16, tag="vh", name="vh")
                nc.sync.dma_start(
                    out=vh.rearrange("p (t n) -> p t n", n=n), in_=v[h]
                )

                # ---- mm1: causal score strips S^T[s-strip j, t>=128j] ----
                strips = [
                    strippool.tile(
                        [128, t - 128 * j], FP16,
                        tag=f"strip{j}", name=f"strip{j}",
                    )
                    for j in range(nt)
                ]
                for j in reversed(range(nt)):
                    tstart = 128 * j
                    lo = tstart
                    while lo < t:
                        hi = min(t, (lo // 512 + 1) * 512)
                        w = hi - lo
                        ps = spool.tile([128, 512], F32, name="s")
                        for c in range(2):
                            nc.tensor.matmul(
                                ps[:, :w],
                                lhsT=qrt[c][:, tstart:tstart + 128],
                                rhs=qrt[c][:, lo:hi],
                                start=(c == 0),
                                stop=(c == 1),
                            )
                        # writeback (cast to fp16; strict mask on diag block)
                        if lo == tstart:
                            nc.vector.tensor_tensor(
                                out=strips[j][:, 0:128],
                                in0=ps[:, 0:128],
                                in1=mask,
                                op=MULT,
                            )
                            if w > 128:
                                wb_copy(strips[j][:, 128:w], ps[:, 128:w])
                        else:
                            wb_copy(
                                strips[j][:, lo - tstart:hi - tstart],
                                ps[:, :w],
                            )
                        lo = hi

                # ---- mm2: O^T[nb] via V-stationary bank passes ----
                for nb in range(2):
                    oht = ohtpool.tile(
                        [128, t], F32, tag=f"oht{nb}", name=f"oht{nb}"
                    )
                    for k in range(nbk):
                        po = opool.tile([128, 512], F32, name="po")
                        for j in range(4 * k + 4):
                            lo = max(128 * j, 512 * k)
                            hi = 512 * (k + 1)
                            nc.tensor.matmul(
                                po[:, lo - 512 * k:512],
                                lhsT=vh[:, j * n + nb * 128:j * n + nb * 128 + 128],
                                rhs=strips[j][:, lo - 128 * j:hi - 128 * j],
                                start=(j == 0),
                                stop=(j == 4 * k + 3),
                            )
                        if nb == 1 and k == nbk - 1:
                            # split the final drain so the tail DMA is short
                            for half in range(2):
                                ksl = slice(
                                    512 * k + 256 * half,
                                    512 * k + 256 * (half + 1),
                                )
                                psl = slice(256 * half, 256 * (half + 1))
                                wb_copy(oht[:, ksl], po[:, psl])
                                eng = nc.scalar if half == 0 else nc.sync
                                eng.dma_start(
                                    out=o[h][nb][:, ksl], in_=oht[:, ksl]
                                )
                        else:
                            ksl = slice(512 * k, 512 * (k + 1))
                            wb_copy(oht[:, ksl], po)
                            nc.scalar.dma_start(
                                out=o[h][nb][:, ksl], in_=oht[:, ksl]
                            )

    if waitsplit:
        _split_overloaded_waits(nc)
    return nc


_NC_CACHE = {}


def get_nc(h_per_core=H_PER_CORE, t=T, n=N):
    key = (h_per_core, t, n)
    if key not in _NC_CACHE:
        _NC_CACHE[key] = build_nc(h_per_core, t, n)
    return _NC_CACHE[key]


def make_in_maps(Q, V, n_cores=N_CORES):
    b, nh, t, n = Q.shape
    h_per_core = (b * nh) // n_cores
    qf = np.asarray(Q, dtype=np.float32).reshape(b * nh, t, n)
    vf = np.asarray(V, dtype=np.float32).reshape(b * nh, t, n)
    # host-side RoPE in f32, single rounding to fp16
    cos, sin_a = rope_tables(t, n, np.float32)
    qsw = np.empty_like(qf)
    qsw[..., 0::2] = qf[..., 1::2]
    qsw[..., 1::2] = qf[..., 0::2]
    qr = (qf * cos + qsw * sin_a).astype(HF)
    # pre-transposed (n, t) layout so the device needs only plain DMAs
    qtb = np.ascontiguousarray(qr.transpose(0, 2, 1))
    # v swizzled to [128, t//128, n]: row s = 128*tb + p  ->  [p, tb, n]
    vb = (
        vf.astype(HF)
        .reshape(b * nh, t // 128, 128, n)
        .transpose(0, 2, 1, 3)
    )
    vb = np.ascontiguousarray(vb)
    in_maps = []
    for c in range(n_cores):
        sl = slice(c * h_per_core, (c + 1) * h_per_core)
        in_maps.append(
            {
                "q": np.ascontiguousarray(qtb[sl]),
                "v": np.ascontiguousarray(vb[sl]),
            }
        )
    return in_maps


def unshard(res, b=B, nh=NH, t=T, n=N, n_cores=N_CORES):
    """Gather per-core O^T outputs [h, 2, 128, t] into full (b, nh, t, n)."""
    outs = [res.results[c]["o"] for c in range(n_cores)]
    ot = np.concatenate(outs, axis=0)  # (b*nh, 2, 128, t)
    out = ot.transpose(0, 3, 1, 2).reshape(b * nh, t, n)
    return np.ascontiguousarray(out).reshape(b, nh, t, n).astype(np.float32)


def kernel(Q, K, V):
    """Full-input entry point: Q, K, V are (B, NH, T, N) float32 numpy arrays.
    K is unused (the module self-keys attention on rotated Q)."""
    Q = np.asarray(Q)
    V = np.asarray(V)
    b, nh, t, n = Q.shape
    nc = get_nc((b * nh) // N_CORES, t, n)
    in_maps = make_in_maps(Q, V, N_CORES)
    res = None
    last_err = None
    for attempt in range(3):  # retry transient device/runtime failures
        try:
            res = run_bass_kernel_spmd(
                nc, in_maps, core_ids=list(range(N_CORES)), trace=False
            )
            break
        except Exception as e:  # e.g. NRT_EXEC_UNIT_UNRECOVERABLE after a
            last_err = e  # wedged prior run; a clean retry usually recovers
            import time as _time

            _time.sleep(2.0 * (attempt + 1))
    if res is None:
        raise last_err
    return unshard(res, b, nh, t, n, N_CORES)
